# revision 25
# baseline (speedup 1.0000x reference)
"""Longhorn SSM layer on 8 Trainium2 cores.

Sharding: core (b, j) with b in {0,1}, j in {0..3} handles batch b and
d_inner channel chunk [j*512, (j+1)*512).  The x_proj contraction needs all
d_inner channels, so partial x_dbl results are AllReduced across the 4 cores
of each batch (split into two L-halves so the collective overlaps phase A).
The final out_proj partials are summed on the host.

v2 notes (vs v1):
  - all matmuls bf16 (fp32 matmuls ran at ~1/6 rate on PE)
  - depthwise conv as PE diagonal-weight matmuls accumulated in PSUM
  - gpsimd evicted from the scan phase (Pool shares an SBUF port with DVE
    and degraded DVE 2x-mode ops ~4x when running concurrently)
  - full-L scans (FD=2048, no inter-half carry), g-outer loop
  - D*x folded into the Y accumulation as a diag(D) matmul
  - z and y*gate stay in SBUF (no DRAM roundtrip)
"""

import numpy as np
import ml_dtypes

import concourse.bacc as bacc
import concourse.bass as bass
import concourse.tile as tile
from concourse import mybir
from concourse.bass_utils import run_bass_kernel_spmd

F32 = mybir.dt.float32
BF16 = mybir.dt.bfloat16
AL = mybir.AluOpType
AF = mybir.ActivationFunctionType

BF = ml_dtypes.bfloat16


def build_module(L, DM, DI, DCH, NST, DTR, num_devices, use_collective):
    NG = DCH // 128          # d-tiles per core (4)
    NK = DM // 128           # K-tiles for in_proj (8)
    NO = DM // 128           # out_proj output tiles (8)
    TQ = 512                 # matmul moving-dim tile
    NTQ = L // TQ            # 4
    LH = L // 2              # collective chunk (1024)
    NR = DTR + 2 * NST       # x_proj rows (96)
    PAD = 3                  # conv left pad

    nc = bacc.Bacc(
        "TRN2",
        target_bir_lowering=False,
        debug=False,
        enable_asserts=False,
        num_devices=num_devices,
    )

    # ---- I/O -------------------------------------------------------------
    hT_d = nc.dram_tensor("hT", [DM, L], BF16, kind="ExternalInput")
    wx_d = nc.dram_tensor("wx", [128, NK * NG * 128], BF16, kind="ExternalInput")
    wz_d = nc.dram_tensor("wz", [128, NK * NG * 128], BF16, kind="ExternalInput")
    wo_d = nc.dram_tensor("wo", [128, NG * NO * 128], BF16, kind="ExternalInput")
    dtw_d = nc.dram_tensor("dtw", [DTR, NG * 128], BF16, kind="ExternalInput")
    xpw_d = nc.dram_tensor("xpw", [128, NG * NR], BF16, kind="ExternalInput")
    cwd_d = nc.dram_tensor("cwd", [128, NG * 4 * 128], BF16, kind="ExternalInput")
    dgd_d = nc.dram_tensor("dgd", [128, NG * 128], BF16, kind="ExternalInput")
    pvec_d = nc.dram_tensor("pvec", [128, NG * 2], F32, kind="ExternalInput")
    ones_d = nc.dram_tensor("ones16", [NST, 128], BF16, kind="ExternalInput")
    id_d = nc.dram_tensor("id128", [128, 128], BF16, kind="ExternalInput")
    outA_d = nc.dram_tensor("outA", [DM, L], F32, kind="ExternalOutput")
    outB_d = nc.dram_tensor("outB", [DM, L], F32, kind="ExternalOutput")

    # internal DRAM
    cc_in = [nc.dram_tensor(f"ccin{h}", [NR, LH], BF16, kind="Internal")
             for h in range(2)]
    cc_out = [nc.dram_tensor(f"ccout{h}", [NR, LH], BF16, kind="Internal")
              for h in range(2)]
    kkbd = nc.dram_tensor("kkbd", [NST, L], BF16, kind="Internal")

    groups = [[0, 1, 2, 3], [4, 5, 6, 7]] if num_devices == 8 else [[0]]

    with tile.TileContext(nc) as tc:
        with (
            tc.tile_pool(name="const", bufs=1) as constp,
            tc.tile_pool(name="persist", bufs=1) as pp,
        ):
            ones_sb = constp.tile([NST, 128], BF16)
            nc.sync.dma_start(ones_sb, ones_d.ap())
            id_sb = constp.tile([128, 128], BF16)
            nc.sync.dma_start(id_sb, id_d.ap())
            dgd_sb = constp.tile([128, NG, 128], BF16)
            nc.sync.dma_start(dgd_sb, dgd_d.ap().rearrange("p (g m) -> p g m", g=NG))
            pvec = constp.tile([128, NG, 2], F32)   # [...,0]=-dtb, [...,1]=conv_b
            nc.sync.dma_start(pvec, pvec_d.ap().rearrange("p (g c) -> p g c", g=NG))

            # persistent SBUF through the scan phase (bf16, 2 bytes)
            xs = pp.tile([128, NG, L], BF16)      # silu(conv(x))
            zb = pp.tile([128, NG, L], BF16)      # z (gate input)
            dtvb = pp.tile([128, NG, L], BF16)    # dtv
            ub = pp.tile([128, NG, L], BF16)      # xs*dtv
            ygb = pp.tile([128, NG, L], BF16)     # (y + D*xs)*silu(z)

            # ---------------- phase A: in_proj / conv / x_dbl ------------
            with (
                tc.tile_pool(name="hw", bufs=1) as hwp,
                tc.tile_pool(name="xpre", bufs=1) as xprep,
                tc.tile_pool(name="psA", bufs=3, space="PSUM") as psA,
                tc.tile_pool(name="psC", bufs=2, space="PSUM") as psCp,
                tc.tile_pool(name="psX", bufs=1, space="PSUM") as psXp,
                tc.tile_pool(name="asm", bufs=3) as asmp,
            ):
                # DMA priority: first-tq activations + x-weights first
                wx_sb = hwp.tile([128, NK, NG, 128], BF16)
                hsbs = []
                for tq in range(NTQ):
                    hsbs.append(hwp.tile([128, NK, TQ], BF16, name=f"hsb{tq}",
                                         tag=f"hsb{tq}"))
                for k in range(NK):
                    nc.sync.dma_start(
                        hsbs[0][:, k], hT_d.ap()[k * 128:(k + 1) * 128, 0:TQ])
                    nc.sync.dma_start(
                        wx_sb[:, k], wx_d.ap()[:, k * NG * 128:(k + 1) * NG * 128]
                        .rearrange("p (g m) -> p g m", g=NG))
                cw_sb = hwp.tile([128, NG, 4, 128], BF16)
                nc.sync.dma_start(
                    cw_sb, cwd_d.ap().rearrange("p (g j m) -> p g j m", g=NG, j=4))
                xpw_sb = hwp.tile([128, NG, NR], BF16)
                for g in range(NG):
                    nc.sync.dma_start(
                        xpw_sb[:, g], xpw_d.ap()[:, g * NR:(g + 1) * NR])
                for tq in range(1, NTQ):
                    ts = slice(tq * TQ, (tq + 1) * TQ)
                    for k in range(NK):
                        nc.sync.dma_start(
                            hsbs[tq][:, k], hT_d.ap()[k * 128:(k + 1) * 128, ts])
                wz_sb = hwp.tile([128, NK, NG, 128], BF16)
                for k in range(NK):
                    nc.sync.dma_start(
                        wz_sb[:, k], wz_d.ap()[:, k * NG * 128:(k + 1) * NG * 128]
                        .rearrange("p (g m) -> p g m", g=NG))

                xpre = xprep.tile([128, NG, L + PAD], BF16)
                for g in range(NG):
                    nc.vector.memset(xpre[:, g, 0:PAD], 0.0)

                # x-side in_proj + conv + x_dbl + collective (critical path);
                # z-side matmuls deferred below the CC triggers
                for h in range(2):
                    for tq2 in range(2):
                        tq = 2 * h + tq2
                        ts = slice(tq * TQ, (tq + 1) * TQ)
                        for g in range(NG):
                            ps = psA.tile([128, TQ], F32, name="ps_xz",
                                          tag="psxz")
                            for k in range(NK):
                                nc.tensor.matmul(ps, wx_sb[:, k, g, :],
                                                 hsbs[tq][:, k, :],
                                                 start=(k == 0),
                                                 stop=(k == NK - 1))
                            nc.scalar.copy(
                                xpre[:, g, PAD + tq * TQ: PAD + (tq + 1) * TQ],
                                ps)
                            # conv: 4 diag-weight matmuls, shifted inputs
                            pc = psCp.tile([128, TQ], F32, name="pc", tag="pc")
                            for j in range(4):
                                nc.tensor.matmul(
                                    pc, cw_sb[:, g, j, :],
                                    xpre[:, g, tq * TQ + j: tq * TQ + j + TQ],
                                    start=(j == 0), stop=(j == 3))
                            # xs = silu(pc + cb), straight from PSUM
                            nc.scalar.activation(xs[:, g, ts], pc, AF.Silu,
                                                 bias=pvec[:, g, 1:2])
                    # partial x_dbl for this half
                    psX = psXp.tile([NR, LH], F32, name="psX", tag="psX")
                    for tq2 in range(2):
                        for g in range(NG):
                            ss = slice(tq2 * TQ, (tq2 + 1) * TQ)
                            nc.tensor.matmul(
                                psX[:, ss], xpw_sb[:, g, :],
                                xs[:, g, h * LH + tq2 * TQ:
                                   h * LH + (tq2 + 1) * TQ],
                                start=(g == 0), stop=(g == NG - 1))
                    xdp = asmp.tile([NR, LH], BF16, name="xdp", tag="xdp",
                                    bufs=2)
                    nc.scalar.copy(xdp, psX)
                    nc.sync.dma_start(cc_in[h].ap(), xdp)
                    if use_collective:
                        nc.gpsimd.collective_compute(
                            "AllReduce", AL.add, replica_groups=groups,
                            ins=[cc_in[h].ap()], outs=[cc_out[h].ap()])
                    else:
                        nc.sync.dma_start(cc_out[h].ap(), cc_in[h].ap())
                # z-side in_proj (needed only at drain time)
                for tq in range(NTQ):
                    ts = slice(tq * TQ, (tq + 1) * TQ)
                    for g in range(NG):
                        psz = psA.tile([128, TQ], F32, name="ps_z", tag="psxz")
                        for k in range(NK):
                            nc.tensor.matmul(psz, wz_sb[:, k, g, :],
                                             hsbs[tq][:, k, :],
                                             start=(k == 0),
                                             stop=(k == NK - 1))
                        nc.scalar.copy(zb[:, g, ts], psz)

            # ---------------- phase A2: dt, dtv, u, rows ------------------
            with (
                tc.tile_pool(name="dtw", bufs=1) as dtwp,
                tc.tile_pool(name="psD", bufs=2, space="PSUM") as psDp,
                tc.tile_pool(name="psK", bufs=1, space="PSUM") as psKp,
                tc.tile_pool(name="rows", bufs=2) as rowp,
                tc.tile_pool(name="dtv", bufs=3) as dtvp,
            ):
                dtw_sb = dtwp.tile([DTR, NG, 128], BF16)
                nc.sync.dma_start(
                    dtw_sb, dtw_d.ap().rearrange("p (g m) -> p g m", g=NG))
                for h in range(2):
                    hs = slice(h * LH, (h + 1) * LH)
                    dtl = rowp.tile([DTR, LH], BF16, name="dtl", tag="dtl")
                    nc.sync.dma_start(dtl, cc_out[h].ap()[0:DTR, :])
                    krow = rowp.tile([NST, LH], BF16, name="krow", tag="krow")
                    nc.sync.dma_start(krow, cc_out[h].ap()[DTR:DTR + NST, :])
                    kk = rowp.tile([NST, LH], F32, name="kk", tag="kk")
                    nc.scalar.activation(kk, krow, AF.Square)
                    kkb16 = rowp.tile([NST, LH], BF16, name="kkb16", tag="kkb16")
                    nc.scalar.copy(kkb16, kk)
                    nc.sync.dma_start(kkbd.ap()[:, hs], kkb16)
                    # SK[t] = sum_n kk, broadcast to 128 partitions
                    psK = psKp.tile([128, LH], F32, name="psK", tag="psK")
                    for s2 in range(2):
                        ss = slice(s2 * TQ, (s2 + 1) * TQ)
                        nc.tensor.matmul(psK[:, ss], ones_sb, kkb16[:, ss],
                                         start=True, stop=True)
                    psKs = rowp.tile([128, LH], BF16, name="psKs", tag="psKs")
                    nc.scalar.copy(psKs, psK)

                    # batched per activation function to avoid ACT table
                    # reloads; dtv = 1/(1 + E + sum kk) = sigmoid(-ln(E + SK))
                    egs, dens, lnws = [], [], []
                    for g in range(NG):
                        psD = psDp.tile([128, LH], F32, name="psD", tag="psD")
                        for s2 in range(2):
                            ss = slice(s2 * TQ, (s2 + 1) * TQ)
                            nc.tensor.matmul(psD[:, ss], dtw_sb[:, g, :],
                                             dtl[:, ss], start=True, stop=True)
                        eg = dtvp.tile([128, LH], BF16, name=f"eg{g}",
                                       tag=f"eg{g}", bufs=1)
                        nc.scalar.activation(eg, psD, AF.Exp,
                                             bias=pvec[:, g, 0:1], scale=-1.0)
                        egs.append(eg)
                    for g in range(NG):
                        den = dtvp.tile([128, LH], BF16, name=f"den{g}",
                                        tag=f"den{g}", bufs=1)
                        nc.vector.tensor_tensor(den, egs[g], psKs, op=AL.add)
                        dens.append(den)
                    for g in range(NG):
                        lnw = dtvp.tile([128, LH], BF16, name=f"lnw{g}",
                                        tag=f"lnw{g}", bufs=1)
                        nc.scalar.activation(lnw, dens[g], AF.Ln)
                        lnws.append(lnw)
                    for g in range(NG):
                        nc.scalar.activation(dtvb[:, g, hs], lnws[g], AF.Sigmoid,
                                             scale=-1.0)
                        nc.vector.tensor_tensor(ub[:, g, hs], xs[:, g, hs],
                                                dtvb[:, g, hs], op=AL.mult)

            # ---------------- phase B: the scan + out_proj ----------------
            # out_proj is split: outA = g0+g1+g2 contributions, computed and
            # DMA'd while g3's scans keep the DVE busy (psY bufs=1 leaves 4
            # PSUM banks for it); outB = g3's contribution in the tail with
            # its PSUM drain on the then-idle DVE.  The host adds outA+outB.
            with (
                tc.tile_pool(name="psY", bufs=1, space="PSUM") as psYp,
                tc.tile_pool(name="psO", bufs=4, space="PSUM") as psOp,
                tc.tile_pool(name="wo", bufs=1) as wop,
                tc.tile_pool(name="bcast", bufs=2) as bcp,
                tc.tile_pool(name="scan", bufs=2) as scp,
                tc.tile_pool(name="drain", bufs=2) as drp,
                tc.tile_pool(name="odr", bufs=3) as odp,
            ):
                wo_sb = wop.tile([128, NG, NO, 128], BF16)
                for g in range(NG):
                    nc.sync.dma_start(
                        wo_sb[:, g],
                        wo_d.ap()[:, g * NO * 128:(g + 1) * NO * 128]
                        .rearrange("p (o m) -> p o m", o=NO))
                for g in range(NG):
                    Y = psYp.tile([128, L], F32, name="Y", tag="Y")
                    for n in range(NST):
                        kkb_t = bcp.tile([128, L], BF16, name="kkb_t", tag="kkb")
                        nc.sync.dma_start(
                            kkb_t, kkbd.ap()[n:n + 1, :].broadcast_to([128, L]))
                        kb_t = bcp.tile([128, L], BF16, name="kb_t", tag="kb")
                        qb_t = bcp.tile([128, L], BF16, name="qb_t", tag="qb")
                        for h in range(2):
                            hs = slice(h * LH, (h + 1) * LH)
                            nc.sync.dma_start(
                                kb_t[:, hs],
                                cc_out[h].ap()[DTR + n:DTR + n + 1, :]
                                .broadcast_to([128, LH]))
                            nc.sync.dma_start(
                                qb_t[:, hs],
                                cc_out[h].ap()[DTR + NST + n:DTR + NST + n + 1, :]
                                .broadcast_to([128, LH]))
                        c_t = scp.tile([128, L], BF16, name="c_t", tag="c")
                        nc.vector.tensor_tensor(c_t, dtvb[:, g, :], kkb_t,
                                                op=AL.mult)
                        a_t = scp.tile([128, L], F32, name="a_t", tag="a")
                        nc.scalar.activation(a_t, c_t, AF.Identity,
                                             bias=1.0, scale=-1.0)
                        b_t = scp.tile([128, L], BF16, name="b_t", tag="b")
                        nc.vector.tensor_tensor(b_t, ub[:, g, :], kb_t,
                                                op=AL.mult)
                        s_t = scp.tile([128, L], BF16, name="s_t", tag="s",
                                       bufs=3)
                        nc.vector.tensor_tensor_scan(
                            s_t, a_t, b_t, 0.0, op0=AL.mult, op1=AL.add)
                        p_t = scp.tile([128, L], BF16, name="p_t", tag="p",
                                       bufs=4)
                        nc.vector.tensor_tensor(p_t, s_t, qb_t, op=AL.mult)
                        for h4 in range(4):
                            nc.tensor.matmul(
                                Y[:, h4 * TQ:(h4 + 1) * TQ],
                                id_sb, p_t[:, h4 * TQ:(h4 + 1) * TQ],
                                start=(n == 0), stop=False)
                    # skip term D*xs folded into the PSUM accumulation
                    for h4 in range(4):
                        nc.tensor.matmul(
                            Y[:, h4 * TQ:(h4 + 1) * TQ],
                            dgd_sb[:, g, :],
                            xs[:, g, h4 * TQ:(h4 + 1) * TQ],
                            start=False, stop=True)
                    # drain: ygb = (y + D*xs) * silu(z)
                    zs2 = drp.tile([128, L], BF16, name="zs2", tag="zs2")
                    nc.scalar.activation(zs2, zb[:, g, :], AF.Silu)
                    nc.vector.tensor_tensor(ygb[:, g, :], Y, zs2, op=AL.mult)
                    if g == 2:
                        # outA = out_proj over g0..g2, hidden under g3 scans
                        for tq in range(NTQ):
                            ts = slice(tq * TQ, (tq + 1) * TQ)
                            for o in range(NO):
                                po = psOp.tile([128, TQ], F32, name="po",
                                               tag="po")
                                for g2 in range(3):
                                    nc.tensor.matmul(po, wo_sb[:, g2, o, :],
                                                     ygb[:, g2, ts],
                                                     start=(g2 == 0),
                                                     stop=(g2 == 2))
                                ot = odp.tile([128, TQ], F32, name="ot",
                                              tag="ot")
                                nc.scalar.copy(ot, po)
                                nc.sync.dma_start(
                                    outA_d.ap()[o * 128:(o + 1) * 128, ts], ot)
                # outB = g3's out_proj contribution (tail; DVE drains PSUM)
                for tq in range(NTQ):
                    ts = slice(tq * TQ, (tq + 1) * TQ)
                    for o in range(NO):
                        po = psOp.tile([128, TQ], F32, name="po", tag="po")
                        nc.tensor.matmul(po, wo_sb[:, 3, o, :], ygb[:, 3, ts],
                                         start=True, stop=True)
                        ot = odp.tile([128, TQ], F32, name="otb", tag="ot")
                        nc.vector.tensor_copy(ot, po)
                        nc.sync.dma_start(
                            outB_d.ap()[o * 128:(o + 1) * 128, ts], ot)

    nc.compile()
    return nc


# ----------------------------------------------------------------------------
# host-side packing
# ----------------------------------------------------------------------------

def pack_core_inputs(inputs, b, j, L, DM, DI, DCH, NST, DTR):
    NG = DCH // 128
    NK = DM // 128
    NO = DM // 128
    NR = DTR + 2 * NST
    ch = slice(j * DCH, (j + 1) * DCH)

    h = np.asarray(inputs["hidden_states"], np.float32)
    ipw = np.asarray(inputs["in_proj_w"], np.float32)
    cw = np.asarray(inputs["conv_w"], np.float32).reshape(DI, 4)
    cb = np.asarray(inputs["conv_b"], np.float32)
    xpw = np.asarray(inputs["x_proj_w"], np.float32)
    dtw = np.asarray(inputs["dt_head_w"], np.float32)
    dtb = np.asarray(inputs["dt_head_b"], np.float32)
    opw = np.asarray(inputs["out_proj_w"], np.float32)
    D = np.asarray(inputs["D"], np.float32)

    hT = np.ascontiguousarray(h[b].T).astype(BF)                        # [DM, L]
    wx = np.ascontiguousarray(
        ipw[ch].T.reshape(NK, 128, NG, 128).transpose(1, 0, 2, 3)
        .reshape(128, NK * NG * 128)).astype(BF)
    wz = np.ascontiguousarray(
        ipw[DI + j * DCH: DI + (j + 1) * DCH].T
        .reshape(NK, 128, NG, 128).transpose(1, 0, 2, 3)
        .reshape(128, NK * NG * 128)).astype(BF)
    wo = np.ascontiguousarray(
        opw[:, ch].T.reshape(NG, 128, NO, 128).transpose(1, 0, 2, 3)
        .reshape(128, NG * NO * 128)).astype(BF)
    dtwp = np.ascontiguousarray(dtw[ch].T.reshape(DTR, NG * 128)).astype(BF)
    xpwp = np.ascontiguousarray(
        xpw[:, ch].T.reshape(NG, 128, NR).transpose(1, 0, 2)
        .reshape(128, NG * NR)).astype(BF)

    # conv taps / D as diagonal matmul weights: cwd[p, g, j, m] = w_j[d] if
    # p == m else 0 (d = local channel g*128+p); dgd likewise with D.
    cwd = np.zeros((128, NG, 4, 128), np.float32)
    dgd = np.zeros((128, NG, 128), np.float32)
    pv = np.zeros((128, NG, 2), np.float32)
    r = np.arange(128)
    for g in range(NG):
        rows = slice(j * DCH + g * 128, j * DCH + (g + 1) * 128)
        cwd[r, g, :, r] = cw[rows]                  # [128, 4]
        dgd[r, g, r] = D[rows]
        pv[:, g, 0] = -dtb[rows]
        pv[:, g, 1] = cb[rows]

    return {
        "hT": hT,
        "wx": wx,
        "wz": wz,
        "wo": wo,
        "dtw": dtwp,
        "xpw": xpwp,
        "cwd": np.ascontiguousarray(cwd.reshape(128, NG * 4 * 128)).astype(BF),
        "dgd": np.ascontiguousarray(dgd.reshape(128, NG * 128)).astype(BF),
        "pvec": np.ascontiguousarray(pv.reshape(128, NG * 2)),
        "ones16": np.ones((NST, 128), np.float32).astype(BF),
        "id128": np.eye(128, dtype=np.float32).astype(BF),
    }


_CACHE = {}


def _get_module(key, *args, **kw):
    if key not in _CACHE:
        _CACHE[key] = build_module(*args, **kw)
    return _CACHE[key]


def run(inputs, trace=False, trace_cores=None):
    L, DM, DI = 2048, 1024, 2048
    DCH, NST, DTR = 512, 16, 64
    nc = _get_module("full", L, DM, DI, DCH, NST, DTR, 8, True)
    in_maps = []
    for core in range(8):
        b, j = divmod(core, 4)
        in_maps.append(pack_core_inputs(inputs, b, j, L, DM, DI, DCH, NST, DTR))
    res = run_bass_kernel_spmd(
        nc, in_maps, core_ids=list(range(8)), trace=trace,
        trace_cores=trace_cores)
    full = np.empty((2, L, DM), np.float32)
    for b in range(2):
        acc = res.results[4 * b]["outA"].astype(np.float64)
        acc = acc + res.results[4 * b]["outB"]
        for j in range(1, 4):
            acc = acc + res.results[4 * b + j]["outA"]
            acc = acc + res.results[4 * b + j]["outB"]
        full[b] = acc.T.astype(np.float32)
    return full, res


def kernel(**inputs) -> np.ndarray:
    out, _ = run(inputs, trace=False)
    return out


# revision 29
# speedup vs baseline: 1.0394x; 1.0394x over previous
"""Longhorn SSM layer on 8 Trainium2 cores.

Sharding: core (b, j) with b in {0,1}, j in {0..3} handles batch b and
d_inner channel chunk [j*512, (j+1)*512).  The x_proj contraction needs all
d_inner channels, so partial x_dbl results are AllReduced across the 4 cores
of each batch (split into two L-halves so the collective overlaps phase A).
The final out_proj partials are summed on the host.

v2 notes (vs v1):
  - all matmuls bf16 (fp32 matmuls ran at ~1/6 rate on PE)
  - depthwise conv as PE diagonal-weight matmuls accumulated in PSUM
  - gpsimd evicted from the scan phase (Pool shares an SBUF port with DVE
    and degraded DVE 2x-mode ops ~4x when running concurrently)
  - full-L scans (FD=2048, no inter-half carry), g-outer loop
  - D*x folded into the Y accumulation as a diag(D) matmul
  - z and y*gate stay in SBUF (no DRAM roundtrip)
"""

import numpy as np
import ml_dtypes

import concourse.bacc as bacc
import concourse.bass as bass
import concourse.tile as tile
from concourse import mybir
from concourse.bass_utils import run_bass_kernel_spmd

F32 = mybir.dt.float32
BF16 = mybir.dt.bfloat16
AL = mybir.AluOpType
AF = mybir.ActivationFunctionType

BF = ml_dtypes.bfloat16


def build_module(L, DM, DI, DCH, NST, DTR, num_devices, use_collective):
    NG = DCH // 128          # d-tiles per core (4)
    NK = DM // 128           # K-tiles for in_proj (8)
    NO = DM // 128           # out_proj output tiles (8)
    TQ = 512                 # matmul moving-dim tile
    NTQ = L // TQ            # 4
    LH = L // 2              # collective chunk (1024)
    NR = DTR + 2 * NST       # x_proj rows (96)
    PAD = 3                  # conv left pad

    nc = bacc.Bacc(
        "TRN2",
        target_bir_lowering=False,
        debug=False,
        enable_asserts=False,
        num_devices=num_devices,
    )

    # ---- I/O -------------------------------------------------------------
    hT_d = nc.dram_tensor("hT", [DM, L], BF16, kind="ExternalInput")
    wx_d = nc.dram_tensor("wx", [128, NK * NG * 128], BF16, kind="ExternalInput")
    wz_d = nc.dram_tensor("wz", [128, NK * NG * 128], BF16, kind="ExternalInput")
    wo_d = nc.dram_tensor("wo", [128, NG * NO * 128], BF16, kind="ExternalInput")
    dtw_d = nc.dram_tensor("dtw", [DTR, NG * 128], BF16, kind="ExternalInput")
    xpw_d = nc.dram_tensor("xpw", [128, NG * NR], BF16, kind="ExternalInput")
    cwd_d = nc.dram_tensor("cwd", [128, NG * 4 * 128], BF16, kind="ExternalInput")
    dgd_d = nc.dram_tensor("dgd", [128, NG * 128], BF16, kind="ExternalInput")
    pvec_d = nc.dram_tensor("pvec", [128, NG * 2], F32, kind="ExternalInput")
    ones_d = nc.dram_tensor("ones16", [NST, 128], BF16, kind="ExternalInput")
    id_d = nc.dram_tensor("id128", [128, 128], BF16, kind="ExternalInput")
    outA_d = nc.dram_tensor("outA", [DM, L], F32, kind="ExternalOutput")
    outB_d = nc.dram_tensor("outB", [DM, L], F32, kind="ExternalOutput")

    # internal DRAM
    cc_in = [nc.dram_tensor(f"ccin{h}", [NR, LH], BF16, kind="Internal")
             for h in range(2)]
    cc_out = [nc.dram_tensor(f"ccout{h}", [NR, LH], BF16, kind="Internal")
              for h in range(2)]
    kbd = nc.dram_tensor("kbd", [NST, L], BF16, kind="Internal")
    qbd = nc.dram_tensor("qbd", [NST, L], BF16, kind="Internal")
    kkbd = nc.dram_tensor("kkbd", [NST, L], BF16, kind="Internal")

    groups = [[0, 1, 2, 3], [4, 5, 6, 7]] if num_devices == 8 else [[0]]

    with tile.TileContext(nc) as tc:
        with (
            tc.tile_pool(name="const", bufs=1) as constp,
            tc.tile_pool(name="persist", bufs=1) as pp,
        ):
            ones_sb = constp.tile([NST, 128], BF16)
            nc.sync.dma_start(ones_sb, ones_d.ap())
            id_sb = constp.tile([128, 128], BF16)
            nc.sync.dma_start(id_sb, id_d.ap())
            dgd_sb = constp.tile([128, NG, 128], BF16)
            nc.sync.dma_start(dgd_sb, dgd_d.ap().rearrange("p (g m) -> p g m", g=NG))
            pvec = constp.tile([128, NG, 2], F32)   # [...,0]=-dtb, [...,1]=conv_b
            nc.sync.dma_start(pvec, pvec_d.ap().rearrange("p (g c) -> p g c", g=NG))

            # persistent SBUF through the scan phase (bf16, 2 bytes)
            xs = pp.tile([128, NG, L], BF16)      # silu(conv(x))
            zb = pp.tile([128, NG, L], BF16)      # z (gate input)
            dtvb = pp.tile([128, NG, L], BF16)    # dtv
            ub = pp.tile([128, NG, L], BF16)      # xs*dtv
            ygb = pp.tile([128, NG, L], BF16)     # (y + D*xs)*silu(z)

            # ---------------- phase A: in_proj / conv / x_dbl ------------
            with (
                tc.tile_pool(name="hw", bufs=1) as hwp,
                tc.tile_pool(name="xpre", bufs=1) as xprep,
                tc.tile_pool(name="psA", bufs=3, space="PSUM") as psA,
                tc.tile_pool(name="psC", bufs=2, space="PSUM") as psCp,
                tc.tile_pool(name="psX", bufs=1, space="PSUM") as psXp,
                tc.tile_pool(name="asm", bufs=3) as asmp,
            ):
                # DMA priority: first-tq activations + x-weights first
                wx_sb = hwp.tile([128, NK, NG, 128], BF16)
                hsbs = []
                for tq in range(NTQ):
                    hsbs.append(hwp.tile([128, NK, TQ], BF16, name=f"hsb{tq}",
                                         tag=f"hsb{tq}"))
                for k in range(NK):
                    nc.sync.dma_start(
                        hsbs[0][:, k], hT_d.ap()[k * 128:(k + 1) * 128, 0:TQ])
                    nc.sync.dma_start(
                        wx_sb[:, k], wx_d.ap()[:, k * NG * 128:(k + 1) * NG * 128]
                        .rearrange("p (g m) -> p g m", g=NG))
                cw_sb = hwp.tile([128, NG, 4, 128], BF16)
                nc.sync.dma_start(
                    cw_sb, cwd_d.ap().rearrange("p (g j m) -> p g j m", g=NG, j=4))
                xpw_sb = hwp.tile([128, NG, NR], BF16)
                for g in range(NG):
                    nc.sync.dma_start(
                        xpw_sb[:, g], xpw_d.ap()[:, g * NR:(g + 1) * NR])
                for tq in range(1, NTQ):
                    ts = slice(tq * TQ, (tq + 1) * TQ)
                    for k in range(NK):
                        nc.sync.dma_start(
                            hsbs[tq][:, k], hT_d.ap()[k * 128:(k + 1) * 128, ts])
                wz_sb = hwp.tile([128, NK, NG, 128], BF16)
                for k in range(NK):
                    nc.sync.dma_start(
                        wz_sb[:, k], wz_d.ap()[:, k * NG * 128:(k + 1) * NG * 128]
                        .rearrange("p (g m) -> p g m", g=NG))

                xpre = xprep.tile([128, NG, L + PAD], BF16)
                for g in range(NG):
                    nc.vector.memset(xpre[:, g, 0:PAD], 0.0)

                # x-side in_proj + conv + x_dbl + collective (critical path);
                # z-side matmuls deferred below the CC triggers
                for h in range(2):
                    for tq2 in range(2):
                        tq = 2 * h + tq2
                        ts = slice(tq * TQ, (tq + 1) * TQ)
                        for g in range(NG):
                            ps = psA.tile([128, TQ], F32, name="ps_xz",
                                          tag="psxz")
                            for k in range(NK):
                                nc.tensor.matmul(ps, wx_sb[:, k, g, :],
                                                 hsbs[tq][:, k, :],
                                                 start=(k == 0),
                                                 stop=(k == NK - 1))
                            nc.scalar.copy(
                                xpre[:, g, PAD + tq * TQ: PAD + (tq + 1) * TQ],
                                ps)
                            # conv: 4 diag-weight matmuls, shifted inputs
                            pc = psCp.tile([128, TQ], F32, name="pc", tag="pc")
                            for j in range(4):
                                nc.tensor.matmul(
                                    pc, cw_sb[:, g, j, :],
                                    xpre[:, g, tq * TQ + j: tq * TQ + j + TQ],
                                    start=(j == 0), stop=(j == 3))
                            # xs = silu(pc + cb), straight from PSUM
                            nc.scalar.activation(xs[:, g, ts], pc, AF.Silu,
                                                 bias=pvec[:, g, 1:2])
                    # partial x_dbl for this half
                    psX = psXp.tile([NR, LH], F32, name="psX", tag="psX")
                    for tq2 in range(2):
                        for g in range(NG):
                            ss = slice(tq2 * TQ, (tq2 + 1) * TQ)
                            nc.tensor.matmul(
                                psX[:, ss], xpw_sb[:, g, :],
                                xs[:, g, h * LH + tq2 * TQ:
                                   h * LH + (tq2 + 1) * TQ],
                                start=(g == 0), stop=(g == NG - 1))
                    xdp = asmp.tile([NR, LH], BF16, name="xdp", tag="xdp",
                                    bufs=2)
                    nc.scalar.copy(xdp, psX)
                    nc.sync.dma_start(cc_in[h].ap(), xdp)
                    if use_collective:
                        nc.gpsimd.collective_compute(
                            "AllReduce", AL.add, replica_groups=groups,
                            ins=[cc_in[h].ap()], outs=[cc_out[h].ap()])
                    else:
                        nc.sync.dma_start(cc_out[h].ap(), cc_in[h].ap())
                # z-side in_proj (needed only at drain time)
                for tq in range(NTQ):
                    ts = slice(tq * TQ, (tq + 1) * TQ)
                    for g in range(NG):
                        psz = psA.tile([128, TQ], F32, name="ps_z", tag="psxz")
                        for k in range(NK):
                            nc.tensor.matmul(psz, wz_sb[:, k, g, :],
                                             hsbs[tq][:, k, :],
                                             start=(k == 0),
                                             stop=(k == NK - 1))
                        nc.scalar.copy(zb[:, g, ts], psz)

            # ---------------- phase A2: dt, dtv, u, rows ------------------
            with (
                tc.tile_pool(name="dtw", bufs=1) as dtwp,
                tc.tile_pool(name="psD", bufs=2, space="PSUM") as psDp,
                tc.tile_pool(name="psK", bufs=1, space="PSUM") as psKp,
                tc.tile_pool(name="rows", bufs=2) as rowp,
                tc.tile_pool(name="dtv", bufs=3) as dtvp,
            ):
                dtw_sb = dtwp.tile([DTR, NG, 128], BF16)
                nc.sync.dma_start(
                    dtw_sb, dtw_d.ap().rearrange("p (g m) -> p g m", g=NG))
                for h in range(2):
                    hs = slice(h * LH, (h + 1) * LH)
                    # stage k/q rows for full-L broadcasts: DRAM -> DRAM
                    nc.sync.dma_start(kbd.ap()[:, hs],
                                      cc_out[h].ap()[DTR:DTR + NST, :])
                    nc.sync.dma_start(qbd.ap()[:, hs],
                                      cc_out[h].ap()[DTR + NST:NR, :])
                    dtl = rowp.tile([DTR, LH], BF16, name="dtl", tag="dtl")
                    nc.sync.dma_start(dtl, cc_out[h].ap()[0:DTR, :])
                    krow = rowp.tile([NST, LH], BF16, name="krow", tag="krow")
                    nc.sync.dma_start(krow, cc_out[h].ap()[DTR:DTR + NST, :])
                    kk = rowp.tile([NST, LH], F32, name="kk", tag="kk")
                    nc.scalar.activation(kk, krow, AF.Square)
                    kkb16 = rowp.tile([NST, LH], BF16, name="kkb16", tag="kkb16")
                    nc.scalar.copy(kkb16, kk)
                    nc.sync.dma_start(kkbd.ap()[:, hs], kkb16)
                    # SK[t] = sum_n kk, broadcast to 128 partitions
                    psK = psKp.tile([128, LH], F32, name="psK", tag="psK")
                    for s2 in range(2):
                        ss = slice(s2 * TQ, (s2 + 1) * TQ)
                        nc.tensor.matmul(psK[:, ss], ones_sb, kkb16[:, ss],
                                         start=True, stop=True)
                    psKs = rowp.tile([128, LH], BF16, name="psKs", tag="psKs")
                    nc.scalar.copy(psKs, psK)

                    # batched per activation function to avoid ACT table
                    # reloads; dtv = 1/(1 + E + sum kk) = sigmoid(-ln(E + SK))
                    egs, dens, lnws = [], [], []
                    for g in range(NG):
                        psD = psDp.tile([128, LH], F32, name="psD", tag="psD")
                        for s2 in range(2):
                            ss = slice(s2 * TQ, (s2 + 1) * TQ)
                            nc.tensor.matmul(psD[:, ss], dtw_sb[:, g, :],
                                             dtl[:, ss], start=True, stop=True)
                        eg = dtvp.tile([128, LH], BF16, name=f"eg{g}",
                                       tag=f"eg{g}", bufs=1)
                        nc.scalar.activation(eg, psD, AF.Exp,
                                             bias=pvec[:, g, 0:1], scale=-1.0)
                        egs.append(eg)
                    for g in range(NG):
                        den = dtvp.tile([128, LH], BF16, name=f"den{g}",
                                        tag=f"den{g}", bufs=1)
                        nc.vector.tensor_tensor(den, egs[g], psKs, op=AL.add)
                        dens.append(den)
                    for g in range(NG):
                        lnw = dtvp.tile([128, LH], BF16, name=f"lnw{g}",
                                        tag=f"lnw{g}", bufs=1)
                        nc.scalar.activation(lnw, dens[g], AF.Ln)
                        lnws.append(lnw)
                    for g in range(NG):
                        nc.scalar.activation(dtvb[:, g, hs], lnws[g], AF.Sigmoid,
                                             scale=-1.0)
                        nc.vector.tensor_tensor(ub[:, g, hs], xs[:, g, hs],
                                                dtvb[:, g, hs], op=AL.mult)

            # ---------------- phase B: the scan + out_proj ----------------
            # out_proj is split: outA = g0+g1+g2 contributions, computed and
            # DMA'd while g3's scans keep the DVE busy (psY bufs=1 leaves 4
            # PSUM banks for it); outB = g3's contribution in the tail with
            # its PSUM drain on the then-idle DVE.  The host adds outA+outB.
            with (
                tc.tile_pool(name="psY", bufs=1, space="PSUM") as psYp,
                tc.tile_pool(name="psO", bufs=4, space="PSUM") as psOp,
                tc.tile_pool(name="wo", bufs=1) as wop,
                tc.tile_pool(name="bcast", bufs=3) as bcp,
                tc.tile_pool(name="scan", bufs=2) as scp,
                tc.tile_pool(name="drain", bufs=2) as drp,
                tc.tile_pool(name="odr", bufs=3) as odp,
            ):
                wo_sb = wop.tile([128, NG, NO, 128], BF16)
                for g in range(NG):
                    nc.sync.dma_start(
                        wo_sb[:, g],
                        wo_d.ap()[:, g * NO * 128:(g + 1) * NO * 128]
                        .rearrange("p (o m) -> p o m", o=NO))
                for g in range(NG):
                    Y = psYp.tile([128, L], F32, name="Y", tag="Y")
                    for n in range(NST):
                        kkb_t = bcp.tile([128, L], BF16, name="kkb_t", tag="kkb")
                        nc.sync.dma_start(
                            kkb_t, kkbd.ap()[n:n + 1, :].broadcast_to([128, L]))
                        kb_t = bcp.tile([128, L], BF16, name="kb_t", tag="kb")
                        nc.sync.dma_start(
                            kb_t, kbd.ap()[n:n + 1, :].broadcast_to([128, L]))
                        qb_t = bcp.tile([128, L], BF16, name="qb_t", tag="qb")
                        nc.sync.dma_start(
                            qb_t, qbd.ap()[n:n + 1, :].broadcast_to([128, L]))
                        c_t = scp.tile([128, L], BF16, name="c_t", tag="c")
                        nc.vector.tensor_tensor(c_t, dtvb[:, g, :], kkb_t,
                                                op=AL.mult)
                        a_t = scp.tile([128, L], F32, name="a_t", tag="a")
                        nc.scalar.activation(a_t, c_t, AF.Identity,
                                             bias=1.0, scale=-1.0)
                        b_t = scp.tile([128, L], BF16, name="b_t", tag="b")
                        nc.vector.tensor_tensor(b_t, ub[:, g, :], kb_t,
                                                op=AL.mult)
                        s_t = scp.tile([128, L], BF16, name="s_t", tag="s",
                                       bufs=3)
                        nc.vector.tensor_tensor_scan(
                            s_t, a_t, b_t, 0.0, op0=AL.mult, op1=AL.add)
                        p_t = scp.tile([128, L], BF16, name="p_t", tag="p",
                                       bufs=4)
                        nc.vector.tensor_tensor(p_t, s_t, qb_t, op=AL.mult)
                        for h4 in range(4):
                            nc.tensor.matmul(
                                Y[:, h4 * TQ:(h4 + 1) * TQ],
                                id_sb, p_t[:, h4 * TQ:(h4 + 1) * TQ],
                                start=(n == 0), stop=False)
                    # skip term D*xs folded into the PSUM accumulation
                    for h4 in range(4):
                        nc.tensor.matmul(
                            Y[:, h4 * TQ:(h4 + 1) * TQ],
                            dgd_sb[:, g, :],
                            xs[:, g, h4 * TQ:(h4 + 1) * TQ],
                            start=False, stop=True)
                    # drain: ygb = (y + D*xs) * silu(z)
                    zs2 = drp.tile([128, L], BF16, name="zs2", tag="zs2")
                    nc.scalar.activation(zs2, zb[:, g, :], AF.Silu)
                    nc.vector.tensor_tensor(ygb[:, g, :], Y, zs2, op=AL.mult)
                    if g == 2:
                        # outA = out_proj over g0..g2, hidden under g3 scans
                        for tq in range(NTQ):
                            ts = slice(tq * TQ, (tq + 1) * TQ)
                            for o in range(NO):
                                po = psOp.tile([128, TQ], F32, name="po",
                                               tag="po")
                                for g2 in range(3):
                                    nc.tensor.matmul(po, wo_sb[:, g2, o, :],
                                                     ygb[:, g2, ts],
                                                     start=(g2 == 0),
                                                     stop=(g2 == 2))
                                ot = odp.tile([128, TQ], F32, name="ot",
                                              tag="ot")
                                nc.scalar.copy(ot, po)
                                nc.sync.dma_start(
                                    outA_d.ap()[o * 128:(o + 1) * 128, ts], ot)
                # outB = g3's out_proj contribution (tail; DVE drains PSUM)
                for tq in range(NTQ):
                    ts = slice(tq * TQ, (tq + 1) * TQ)
                    for o in range(NO):
                        po = psOp.tile([128, TQ], F32, name="po", tag="po")
                        nc.tensor.matmul(po, wo_sb[:, 3, o, :], ygb[:, 3, ts],
                                         start=True, stop=True)
                        ot = odp.tile([128, TQ], F32, name="otb", tag="ot")
                        nc.vector.tensor_copy(ot, po)
                        nc.sync.dma_start(
                            outB_d.ap()[o * 128:(o + 1) * 128, ts], ot)

    nc.compile()
    return nc


# ----------------------------------------------------------------------------
# host-side packing
# ----------------------------------------------------------------------------

def pack_core_inputs(inputs, b, j, L, DM, DI, DCH, NST, DTR):
    NG = DCH // 128
    NK = DM // 128
    NO = DM // 128
    NR = DTR + 2 * NST
    ch = slice(j * DCH, (j + 1) * DCH)

    h = np.asarray(inputs["hidden_states"], np.float32)
    ipw = np.asarray(inputs["in_proj_w"], np.float32)
    cw = np.asarray(inputs["conv_w"], np.float32).reshape(DI, 4)
    cb = np.asarray(inputs["conv_b"], np.float32)
    xpw = np.asarray(inputs["x_proj_w"], np.float32)
    dtw = np.asarray(inputs["dt_head_w"], np.float32)
    dtb = np.asarray(inputs["dt_head_b"], np.float32)
    opw = np.asarray(inputs["out_proj_w"], np.float32)
    D = np.asarray(inputs["D"], np.float32)

    hT = np.ascontiguousarray(h[b].T).astype(BF)                        # [DM, L]
    wx = np.ascontiguousarray(
        ipw[ch].T.reshape(NK, 128, NG, 128).transpose(1, 0, 2, 3)
        .reshape(128, NK * NG * 128)).astype(BF)
    wz = np.ascontiguousarray(
        ipw[DI + j * DCH: DI + (j + 1) * DCH].T
        .reshape(NK, 128, NG, 128).transpose(1, 0, 2, 3)
        .reshape(128, NK * NG * 128)).astype(BF)
    wo = np.ascontiguousarray(
        opw[:, ch].T.reshape(NG, 128, NO, 128).transpose(1, 0, 2, 3)
        .reshape(128, NG * NO * 128)).astype(BF)
    dtwp = np.ascontiguousarray(dtw[ch].T.reshape(DTR, NG * 128)).astype(BF)
    xpwp = np.ascontiguousarray(
        xpw[:, ch].T.reshape(NG, 128, NR).transpose(1, 0, 2)
        .reshape(128, NG * NR)).astype(BF)

    # conv taps / D as diagonal matmul weights: cwd[p, g, j, m] = w_j[d] if
    # p == m else 0 (d = local channel g*128+p); dgd likewise with D.
    cwd = np.zeros((128, NG, 4, 128), np.float32)
    dgd = np.zeros((128, NG, 128), np.float32)
    pv = np.zeros((128, NG, 2), np.float32)
    r = np.arange(128)
    for g in range(NG):
        rows = slice(j * DCH + g * 128, j * DCH + (g + 1) * 128)
        cwd[r, g, :, r] = cw[rows]                  # [128, 4]
        dgd[r, g, r] = D[rows]
        pv[:, g, 0] = -dtb[rows]
        pv[:, g, 1] = cb[rows]

    return {
        "hT": hT,
        "wx": wx,
        "wz": wz,
        "wo": wo,
        "dtw": dtwp,
        "xpw": xpwp,
        "cwd": np.ascontiguousarray(cwd.reshape(128, NG * 4 * 128)).astype(BF),
        "dgd": np.ascontiguousarray(dgd.reshape(128, NG * 128)).astype(BF),
        "pvec": np.ascontiguousarray(pv.reshape(128, NG * 2)),
        "ones16": np.ones((NST, 128), np.float32).astype(BF),
        "id128": np.eye(128, dtype=np.float32).astype(BF),
    }


_CACHE = {}


def _get_module(key, *args, **kw):
    if key not in _CACHE:
        _CACHE[key] = build_module(*args, **kw)
    return _CACHE[key]


def run(inputs, trace=False, trace_cores=None):
    L, DM, DI = 2048, 1024, 2048
    DCH, NST, DTR = 512, 16, 64
    nc = _get_module("full", L, DM, DI, DCH, NST, DTR, 8, True)
    in_maps = []
    for core in range(8):
        b, j = divmod(core, 4)
        in_maps.append(pack_core_inputs(inputs, b, j, L, DM, DI, DCH, NST, DTR))
    res = run_bass_kernel_spmd(
        nc, in_maps, core_ids=list(range(8)), trace=trace,
        trace_cores=trace_cores)
    full = np.empty((2, L, DM), np.float32)
    for b in range(2):
        acc = res.results[4 * b]["outA"].astype(np.float64)
        acc = acc + res.results[4 * b]["outB"]
        for j in range(1, 4):
            acc = acc + res.results[4 * b + j]["outA"]
            acc = acc + res.results[4 * b + j]["outB"]
        full[b] = acc.T.astype(np.float32)
    return full, res


def kernel(**inputs) -> np.ndarray:
    out, _ = run(inputs, trace=False)
    return out
